# revision 60
# baseline (speedup 1.0000x reference)
"""Trainium2 Bass kernel for nn_Attention2D (B=8, C=256, H=W=32, 8 heads, d=32).

Strategy: data-parallel over batch, one batch element per NeuronCore (8 cores).

Per-core pipeline (n = H*W = 1024 tokens, head dim d = 32):
  phase 0: load x [256,1024] fp32 -> bf16 (one cast on DVE, one on ACT);
           load host-prepped weights.
  qkv:     k = w_k @ x, q = (scale*w_q) @ x  ([256,1024] head-major, bf16,
           quad-0 chunks first so sim can start early)
  vpack:   vt[jc] = x[:, jc]^T @ w_v^T packed per head as
           [v(16)|ones(16)|v(16)|ones(16)] -> [128, 8*64] bf16. The ones
           columns make the AV matmul emit the softmax denominator for free.
  sim^T:   per (head, j-chunk): matmul(lhsT=k slice [32,128], rhs=q slice
           [32,512]) -> PSUM ring tiles [128,1536] (3 units); 4 heads
           coreside via row quadrants (K=32 packing).
  exp:     ACT Exp over the 3-unit PSUM tiles -> bf16 SBUF (max-subtraction
           skipped: logits ~N(0,0.8), measured max |sim| < 5; exp safe).
  AV+den:  per (pair, ih, jc): 2 matmuls (head A rows 0:64 at tile_position
           (0,0), head B rows 64:128 at (0,64)); lhsT [128,64] =
           [v(16)|ones(16)|v(16)|ones(16)] so every 32-row quadrant holds
           16 out rows + 16 denominator rows. AV is interleaved into the
           sim loop (lag 3 j-chunks) to fill PE gaps while ACT paces the
           sim ring.
  norm:    rc = reciprocal_approx_fast(acc) (full tile); stream_shuffle
           aligns 1/den onto the out rows; one full-tile multiply writes
           out_all. Junk rows become den/den ~ 1.0 and are killed by the
           zero rows of the padded projection weights.
  proj:    final = w_outT_padded^T @ out_all + b_out -> y [256,1024] fp32.
"""

import numpy as np
import ml_dtypes

B, DIM, H, W = 8, 256, 32, 32
NUM_HEADS = 8
DIM_HEAD = 256
D = DIM_HEAD // NUM_HEADS          # 32 per-head dim
N = H * W                          # 1024 tokens
SCALE = (DIM_HEAD / NUM_HEADS) ** (-0.5)
NCORES = 8

_BF16 = ml_dtypes.bfloat16

_PROGRAM = None  # compiled Bass program cache (one per process)

# DVE exp offload: logits are pre-scaled by 1/EXPN (folded into w_q on the
# host); the ACT engine recovers exp(x) via its free affine (scale=EXPN) and
# the DVE computes exp(x) ~ (1 + x/8192)^8192 in TWO chained custom ops:
# (1+s)^128 (1 add + 7 squarings, exactly the 8-stage v3 pipeline) followed
# by y^64 (6 squarings). Max rel err 0.3% over |x|<=6 -- bf16-noise level.
EXPN = 8192.0
_EXP_OPS = None


def _register_exp_ops():
    """Register custom-DVE ops EXP128_ANT ((1+x)^128) and POW64_ANT (x^64)."""
    global _EXP_OPS
    if _EXP_OPS is not None:
        return _EXP_OPS
    import concourse.dve_ops as dve_ops_mod
    from concourse.dve_spec import Spec, Src0, One, lower
    from concourse.dve_uop import DveOpSpec
    from concourse.dve_ops import DveOp

    def _make(name, body, ref):
        spec = Spec(body=body, reference=ref)
        if name in dve_ops_mod._SUB_OPCODE_FOR_NAME:
            return next(op for op in dve_ops_mod.OPS if op.name == name)
        row = dve_ops_mod._CUSTOM_DVE_ROW_BASE + len(dve_ops_mod.OPS)
        dve_ops_mod._SUB_OPCODE_FOR_NAME[name] = row
        shas = {}
        for ver in ("v3", "v4"):
            uops = lower(spec, ver=ver)
            shas[ver] = DveOpSpec(name=name, opcode=row, uops=uops,
                                  rd1_en=False).sha(ver)
        op = DveOp(name, spec, subdim=False, uops_sha=shas)
        dve_ops_mod.OPS.append(op)
        dve_ops_mod.CUSTOM_DVE_SPECS[name] = spec
        return op

    y1 = Src0 + One
    for _ in range(7):
        y1 = y1 * y1

    def _ref1(in0, in1, c0, c1, c2):
        y = 1.0 + in0
        for _ in range(7):
            y = y * y
        return y

    y2 = Src0
    for _ in range(6):
        y2 = y2 * y2

    def _ref2(in0, in1, c0, c1, c2):
        y = in0
        for _ in range(6):
            y = y * y
        return y

    _EXP_OPS = (_make("EXP128_ANT", y1, _ref1), _make("POW64_ANT", y2, _ref2))
    return _EXP_OPS

# stream_shuffle operates within each 32-partition quadrant (same mask for
# all quadrants). AV lhsT is interleaved [v(16)|ones(16)|v(16)|ones(16)] per
# head, so every quadrant is [out rows 0:16 | den rows 16:32]; the mask pulls
# each quadrant's 1/den rows onto its out rows (and keeps them at 16:32, so
# junk rows become den/den ~ 1).
_RC_SHUF = [16 + i for i in range(16)] + [16 + i for i in range(16)]


def build_kernel_body(tc, y_ap, x_ap, woutT_ap, bout_ap):
    """Emit the per-core attention program into TileContext tc.

    DRAM tensors:
      x_ap:     [256, 1792] bf16   (fused [w_qkv^T | x] per channel chunk:
                                    cols 0:768 = w_qkv^T with the q-part
                                    pre-scaled by SCALE/EXPN, cols 768:1792
                                    = one batch element channels x tokens,
                                    host-cast bf16)
      woutT_ap: [512, 256]  bf16   (w_out^T padded: 64-row blocks per head,
                                    interleaved [w(16)|0(16)|w(16)|0(16)])
      bout_ap:  [256, 1]    fp32
      y_ap:     [256, 1024] fp32 out
    """
    from contextlib import ExitStack
    from concourse import mybir

    nc = tc.nc
    f32 = mybir.dt.float32
    bf16 = mybir.dt.bfloat16

    with ExitStack() as ctx:
        singles = ctx.enter_context(tc.tile_pool(name="singles", bufs=1))
        evac = ctx.enter_context(tc.tile_pool(name="evac", bufs=2))
        exp_pool = ctx.enter_context(tc.tile_pool(name="exp", bufs=16))
        rc_pool = ctx.enter_context(tc.tile_pool(name="rc", bufs=3))
        sim_psum = ctx.enter_context(tc.tile_pool(name="simp", bufs=2, space="PSUM"))
        acc_psum = ctx.enter_context(tc.tile_pool(name="accp", bufs=2, space="PSUM"))

        # ---- phase 0: loads + constant prep ----
        # x (host-cast bf16) and wqkvT ride in ONE fused DMA per 128-channel
        # chunk: [128, 1024 x | 768 wq] -- halves the number of critical-path
        # DMA issues at kernel start.
        # the two chunks are issued from different HWDGE engines (SP and
        # ACT) so their transfers run on separate queues in parallel.
        xw = []
        for c in range(2):
            txw = singles.tile([128, N + 768], bf16, tag=f"xw_{c}")
            eng = nc.sync if c == 0 else nc.scalar
            eng.dma_start(out=txw, in_=x_ap[c * 128:(c + 1) * 128, :])
            xw.append(txw)

        def XB(c, lo, hi):      # x slice, channels c*128.., tokens lo:hi
            return xw[c][:, 768 + lo:768 + hi]

        def WQ(c, lo, hi):      # wqkvT slice, o-channels lo:hi
            return xw[c][:, lo:hi]

        wo = []
        for t in range(4):
            tw = singles.tile([128, 256], bf16, tag=f"wo_{t}")
            nc.sync.dma_start(out=tw, in_=woutT_ap[t * 128:(t + 1) * 128, :])
            wo.append(tw)
        bias = []
        for oc in range(2):
            tb = singles.tile([128, 1], f32, tag=f"bias_{oc}")
            nc.sync.dma_start(out=tb, in_=bout_ap[oc * 128:(oc + 1) * 128, :])
            bias.append(tb)

        # warmup during the ~7us input-DMA wait: a burst of matmuls on
        # memset data starts the PE p-state clock ramp, and a dummy exp
        # pulls the ~1.5-2.7us ACT spline-table load off the critical path.
        warm = singles.tile([128, 512], bf16, tag="warm")
        nc.gpsimd.memset(warm, 0.0)
        wps = acc_psum.tile([128, 512], f32, tag="acc", name="warmps")
        for i in range(6):
            nc.tensor.matmul(wps, warm[:, 0:128], warm,
                             start=(i == 0), stop=(i == 5))
        wexp = singles.tile([128, 8], bf16, tag="warmexp")
        nc.scalar.activation(out=wexp, in_=warm[:, 0:8],
                             func=mybir.ActivationFunctionType.Exp,
                             scale=EXPN)

        # vpack tiles: per jc, [128, 8*64] bf16. memset 1.0; v cols written by
        # strided copies from the vt GEMM. Column block for head h:
        # [v dims 0:16 | ones x16 | v dims 16:32 | ones x16].
        vpack = []
        for jc in range(8):
            tv = singles.tile([128, 512], bf16, tag=f"vpack_{jc}")
            nc.gpsimd.memset(tv, 1.0)
            vpack.append(tv)

        # out_all: final-GEMM rhs, 4 pair tiles x [128, 1024] bf16.
        # pair p = heads (2p, 2p+1): head A rows 0:64, head B rows 64:128,
        # each 64-block interleaved [out(16)|junk(16)|out(16)|junk(16)];
        # junk rows ~1.0 (den * 1/den) are killed by woutT's zero rows.
        out_all = []
        for t in range(4):
            ta = singles.tile([128, N], bf16, tag=f"out_all_{t}")
            out_all.append(ta)

        # ---- qkv GEMM helper: one o-chunk (q: oc 0,1; k: oc 2,3) ----
        qk = [None] * 4

        def emit_qkv(oc):
            dst = singles.tile([128, N], bf16, tag=f"qk_{oc}")
            for nh in range(2):
                ps = acc_psum.tile([128, 512], f32, tag="acc")
                for kc in range(2):
                    nc.tensor.matmul(
                        ps,
                        WQ(kc, oc * 128, (oc + 1) * 128),
                        XB(kc, nh * 512, (nh + 1) * 512),
                        start=(kc == 0),
                        stop=(kc == 1),
                    )
                nc.vector.tensor_copy(out=dst[:, nh * 512:(nh + 1) * 512], in_=ps)
            qk[oc] = dst

        # ---- vT GEMM + packing: vt[jc] = x[:, jc]^T @ w_v^T ----
        def emit_vt(jc):
            ps = acc_psum.tile([128, 256], f32, tag="acc")
            for kc in range(2):
                nc.tensor.matmul(
                    ps,
                    XB(kc, jc * 128, (jc + 1) * 128),
                    WQ(kc, 512, 768),
                    start=(kc == 0),
                    stop=(kc == 1),
                )
            pv = ps[:, :].rearrange("p (h c) -> p h c", c=32)
            vv = vpack[jc][:, :].rearrange("p (h c) -> p h c", c=64)
            nc.vector.tensor_copy(out=vv[:, :, 0:16], in_=pv[:, :, 0:16])
            nc.vector.tensor_copy(out=vv[:, :, 32:48], in_=pv[:, :, 16:32])

        # quad-0 weights first so sim(Q=0) starts ASAP; vt and the quad-1
        # weights are emitted INSIDE group 1 (after its first sim tiles) so
        # the ACT exp stream starts ~5us earlier. The acc-pool psum they use
        # is free until group 1's first AV allocation (at its jc=3).
        emit_qkv(2)   # k heads 0-3
        emit_qkv(0)   # q heads 0-3

        exp_op1, exp_op2 = _register_exp_ops()

        # projection helper: one (oc, nh) unit = 4 accumulating matmuls +
        # bias + store. nh=0 runs early (after group 2); nh=1 in the tail.
        def emit_proj(oc, nh, ps=None):
            if ps is None:
                ps = acc_psum.tile([128, 512], f32, tag="acc")
            for t in range(4):
                nc.tensor.matmul(
                    ps,
                    wo[t][:, oc * 128:(oc + 1) * 128],
                    out_all[t][:, nh * 512:(nh + 1) * 512],
                    start=(t == 0),
                    stop=(t == 3),
                )
            ys = evac.tile([128, 512], bf16, tag="y")
            nc.vector.tensor_scalar_add(out=ys, in0=ps, scalar1=bias[oc])
            nc.sync.dma_start(
                out=y_ap[oc * 128:(oc + 1) * 128, nh * 512:(nh + 1) * 512],
                in_=ys,
            )

        # ---- main loop: sim^T -> exp -> AV(+den) -> normalize ----
        # group g = (Q, ih); 32 units of [128,512] per group; exp tiles hold
        # 3 units (final tile of a group holds 2). Groups are SOFTWARE
        # PIPELINED: group g's AV tail (jc 5-7) and normalize are emitted
        # inside group g+1's jc loop, so PE never drains at group ends while
        # ACT finishes the last exp tiles.
        GROUPS = ((0, 0), (1, 0), (0, 1), (1, 1))
        g_exp = [dict() for _ in GROUPS]    # unit u -> (exp_tile, slot)
        g_acc = [dict() for _ in GROUPS]    # pq -> acc psum tile

        def emit_av(gi, jc):
            Q, ih = GROUPS[gi]
            for pq in range(2):
                if jc == 0:
                    g_acc[gi][pq] = acc_psum.tile(
                        [128, 512], f32, tag="acc", name=f"av_{gi}_{pq}")
                at = g_acc[gi][pq]
                for ab in range(2):
                    hq = 2 * pq + ab
                    hg = 4 * Q + hq
                    et, s = g_exp[gi][jc * 4 + hq]
                    nc.tensor.matmul(
                        at[64 * ab:64 * ab + 64, :],
                        vpack[jc][:, 64 * hg:64 * hg + 64],
                        et[:, s * 512:(s + 1) * 512],
                        start=(jc == 0),
                        stop=(jc == 7),
                        tile_position=(0, 64 * ab),
                        skip_group_check=True,
                    )

        def emit_norm(gi):
            Q, ih = GROUPS[gi]
            for pq in range(2):
                pair = 2 * Q + pq
                at = g_acc[gi][pq]
                rc = rc_pool.tile([128, 512], f32, tag="rc",
                                  name=f"rc_{pair}_{ih}")
                nc.vector.reciprocal_approx_fast(out=rc, in_=at[:, :])
                rca = rc_pool.tile([128, 512], f32, tag="rca",
                                   name=f"rca_{pair}_{ih}")
                nc.vector.stream_shuffle(out=rca, in_=rc, mask=_RC_SHUF)
                nc.vector.tensor_mul(
                    out=out_all[pair][:, ih * 512:(ih + 1) * 512],
                    in0=at[:, :], in1=rca)

        for gi, (Q, ih) in enumerate(GROUPS):
            cur = {"psum": None, "exp": None, "units": 0, "start_u": 0}

            def flush():
                if cur["psum"] is None:
                    return
                w = cur["units"] * 512
                # two mid-group tiles go to the DVE via the 2-pass
                # (1+s)^8192 chain; the rest to ACT (exact exp, recovering
                # x via the free affine scale). Group 0's tile grid is
                # shifted by its 4 single-unit warmup flushes.
                if cur["start_u"] in ((10, 22) if gi == 0 else (9, 21)):
                    scr = rc_pool.tile([128, 1536], f32, tag="escr",
                                       name=f"escr_{gi}_{cur['start_u']}")
                    nc.vector._custom_dve(
                        exp_op1,
                        out=scr[:, 0:w],
                        in0=cur["psum"][:, 0:w],
                    )
                    nc.vector._custom_dve(
                        exp_op2,
                        out=cur["exp"][:, 0:w],
                        in0=scr[:, 0:w],
                    )
                else:
                    nc.scalar.activation(
                        out=cur["exp"][:, 0:w],
                        in_=cur["psum"][:, 0:w],
                        func=mybir.ActivationFunctionType.Exp,
                        scale=EXPN,
                    )
                cur["psum"] = None
                cur["exp"] = None
                cur["units"] = 0

            for jc in range(8):
                if gi == 0 and jc == 3:
                    # PE is 2 tiles ahead of ACT here -- spend the slack
                    # on vt + quad-1 qkv weights. These must ALL precede
                    # group 0's first AV allocation: an AV accumulator
                    # holds its acc-ring slot until normalize, so any
                    # later vt/qkv psum request would deadlock the ring.
                    for vjc in range(8):
                        emit_vt(vjc)
                    emit_qkv(3)   # k heads 4-7
                    emit_qkv(1)   # q heads 4-7
                for hq in range(4):
                    u = jc * 4 + hq
                    if cur["psum"] is None:
                        cur["psum"] = sim_psum.tile(
                            [128, 1536], f32, tag="sim",
                            name=f"sim_{gi}_{u}")
                        cur["exp"] = exp_pool.tile(
                            [128, 1536], bf16, tag="exp",
                            name=f"exp_{gi}_{u}")
                        cur["start_u"] = u
                    s = cur["units"]
                    nc.tensor.matmul(
                        cur["psum"][:, s * 512:(s + 1) * 512],
                        qk[2 + Q][32 * hq:32 * (hq + 1),
                                  jc * 128:(jc + 1) * 128],
                        qk[Q][32 * hq:32 * (hq + 1),
                              ih * 512:(ih + 1) * 512],
                        start=True,
                        stop=True,
                        tile_position=(32 * hq, 0),
                    )
                    g_exp[gi][u] = (cur["exp"], s)
                    cur["units"] += 1
                    # group 0's first jc flushes per unit so ACT spins up
                    # ~3us earlier; afterwards tiles hold 3 units.
                    if cur["units"] == 3 or u == 31 or (gi == 0 and jc == 0):
                        flush()
                # previous group's AV tail + normalize, pipelined into
                # this group's first jc slots
                if gi > 0 and jc <= 2:
                    emit_av(gi - 1, jc + 5)
                    if gi == 3 and jc <= 1:
                        # nh=0 projection: its out_all inputs (groups 0-1)
                        # finished long ago. One unit per jc slot (one
                        # sim-ring steal at a time), emitted BEFORE the
                        # norm chain so its bias-add isn't queued behind
                        # it on DVE; stores overlap the rest of group 3.
                        pp = sim_psum.tile([128, 1536], f32, tag="sim",
                                           name=f"projp_{jc}")
                        emit_proj(jc, 0, pp[:, 0:512])
                    if jc == 2:
                        emit_norm(gi - 1)
                if jc >= 3:
                    emit_av(gi, jc - 3)

        # ---- drain the pipeline: last group's AV tail + normalize ----
        for jc in range(5, 8):
            emit_av(3, jc)
        emit_norm(3)

        # ---- tail projection (nh=1; nh=0 ran early during group 3) ----
        # psum borrows idle sim-ring slots: the acc ring is still held by
        # group 3's AV accumulators until the normalize muls run, which
        # would serialize proj behind the whole DVE chain. On the sim ring
        # the t=0,1 accumulation steps (inputs ready since group 3's jc2)
        # overlap the normalize instead.
        for oc in range(2):
            pp = sim_psum.tile([128, 1536], f32, tag="sim",
                               name=f"projt_{oc}")
            emit_proj(oc, 1, pp[:, 0:512])


def _prep_weights(w_qkv, w_out, b_out):
    """Host-side weight preparation (numpy)."""
    wq = w_qkv.astype(np.float32).copy()
    # fold softmax scale AND the 1/EXPN logit pre-scale into w_q
    wq[0:DIM_HEAD] *= SCALE / EXPN
    wqkvT = np.ascontiguousarray(wq.T).astype(_BF16)          # [256, 768]

    w_outT = np.ascontiguousarray(w_out.astype(np.float32).T)  # [hd, o]
    # interleaved pad matching the AV lhsT layout: per head-block of 64 rows,
    # [w dims 0:16 | zeros x16 | w dims 16:32 | zeros x16]
    pad = np.zeros((8, 4, 16, DIM), dtype=np.float32)
    for h in range(NUM_HEADS):
        pad[h, 0, :, :] = w_outT[h * D:h * D + 16, :]
        pad[h, 2, :, :] = w_outT[h * D + 16:h * D + 32, :]
    woutT = pad.reshape(512, DIM).astype(_BF16)               # [512, 256]

    bout = b_out.astype(np.float32).reshape(DIM, 1)           # [256, 1]
    return wqkvT, woutT, bout


def _build_program():
    global _PROGRAM
    if _PROGRAM is not None:
        return _PROGRAM
    import concourse.tile as tile
    from concourse import bacc, mybir

    nc = bacc.Bacc("TRN2", target_bir_lowering=False, debug=False,
                   num_devices=NCORES)
    xw_ap = nc.dram_tensor("xw", [DIM, N + 3 * DIM_HEAD], mybir.dt.bfloat16,
                           kind="ExternalInput").ap()
    woutT_ap = nc.dram_tensor("woutT", [512, DIM], mybir.dt.bfloat16,
                              kind="ExternalInput").ap()
    bout_ap = nc.dram_tensor("bout", [DIM, 1], mybir.dt.float32,
                             kind="ExternalInput").ap()
    y_ap = nc.dram_tensor("y", [DIM, N], mybir.dt.bfloat16,
                          kind="ExternalOutput").ap()
    with tile.TileContext(nc) as tc:
        build_kernel_body(tc, y_ap, xw_ap, woutT_ap, bout_ap)
    nc.compile()
    _PROGRAM = nc
    return nc


def kernel(x, w_qkv, w_out, b_out, trace=False):
    """Full-input entry point: shard over batch, run on 8 cores, gather."""
    from concourse import bass_utils

    nc = _build_program()
    wqkvT, woutT, bout = _prep_weights(w_qkv, w_out, b_out)
    in_maps = []
    for b in range(B):
        xb16 = np.asarray(x[b], dtype=np.float32).reshape(DIM, N).astype(_BF16)
        in_maps.append({
            "xw": np.ascontiguousarray(np.concatenate([wqkvT, xb16], axis=1)),
            "woutT": woutT,
            "bout": bout,
        })
    res = bass_utils.run_bass_kernel_spmd(
        nc, in_maps, core_ids=list(range(NCORES)), trace=trace)
    y = np.stack([np.asarray(res.results[b]["y"], dtype=np.float32)
                  .reshape(DIM, H, W) for b in range(B)])
    kernel.last_results = res
    return y


# revision 64
# speedup vs baseline: 1.1468x; 1.1468x over previous
"""Trainium2 Bass kernel for nn_Attention2D (B=8, C=256, H=W=32, 8 heads, d=32).

Strategy: data-parallel over batch, one batch element per NeuronCore (8 cores).

Per-core pipeline (n = H*W = 1024 tokens, head dim d = 32):
  phase 0: load x [256,1024] fp32 -> bf16 (one cast on DVE, one on ACT);
           load host-prepped weights.
  qkv:     k = w_k @ x, q = (scale*w_q) @ x  ([256,1024] head-major, bf16,
           quad-0 chunks first so sim can start early)
  vpack:   vt[jc] = x[:, jc]^T @ w_v^T packed per head as
           [v(16)|ones(16)|v(16)|ones(16)] -> [128, 8*64] bf16. The ones
           columns make the AV matmul emit the softmax denominator for free.
  sim^T:   per (head, j-chunk): matmul(lhsT=k slice [32,128], rhs=q slice
           [32,512]) -> PSUM ring tiles [128,1536] (3 units); 4 heads
           coreside via row quadrants (K=32 packing).
  exp:     ACT Exp over the 3-unit PSUM tiles -> bf16 SBUF (max-subtraction
           skipped: logits ~N(0,0.8), measured max |sim| < 5; exp safe).
  AV+den:  per (pair, ih, jc): 2 matmuls (head A rows 0:64 at tile_position
           (0,0), head B rows 64:128 at (0,64)); lhsT [128,64] =
           [v(16)|ones(16)|v(16)|ones(16)] so every 32-row quadrant holds
           16 out rows + 16 denominator rows. AV is interleaved into the
           sim loop (lag 3 j-chunks) to fill PE gaps while ACT paces the
           sim ring.
  norm:    rc = reciprocal_approx_fast(acc) (full tile); stream_shuffle
           aligns 1/den onto the out rows; one full-tile multiply writes
           out_all. Junk rows become den/den ~ 1.0 and are killed by the
           zero rows of the padded projection weights.
  proj:    final = w_outT_padded^T @ out_all + b_out -> y [256,1024] fp32.
"""

import numpy as np
import ml_dtypes

B, DIM, H, W = 8, 256, 32, 32
NUM_HEADS = 8
DIM_HEAD = 256
D = DIM_HEAD // NUM_HEADS          # 32 per-head dim
N = H * W                          # 1024 tokens
SCALE = (DIM_HEAD / NUM_HEADS) ** (-0.5)
NCORES = 8

_BF16 = ml_dtypes.bfloat16

_PROGRAM = None  # compiled Bass program cache (one per process)

# DVE exp offload: logits are pre-scaled by 1/EXPN (folded into w_q on the
# host); the ACT engine recovers exp(x) via its free affine (scale=EXPN) and
# the DVE computes exp(x) ~ (1 + x/8192)^8192 in TWO chained custom ops:
# (1+s)^128 (1 add + 7 squarings, exactly the 8-stage v3 pipeline) followed
# by y^64 (6 squarings). Max rel err 0.3% over |x|<=6 -- bf16-noise level.
EXPN = 8192.0
_EXP_OPS = None


def _register_exp_ops():
    """Register custom-DVE ops EXP128_ANT ((1+x)^128) and POW64_ANT (x^64)."""
    global _EXP_OPS
    if _EXP_OPS is not None:
        return _EXP_OPS
    import concourse.dve_ops as dve_ops_mod
    from concourse.dve_spec import Spec, Src0, One, lower
    from concourse.dve_uop import DveOpSpec
    from concourse.dve_ops import DveOp

    def _make(name, body, ref):
        spec = Spec(body=body, reference=ref)
        if name in dve_ops_mod._SUB_OPCODE_FOR_NAME:
            return next(op for op in dve_ops_mod.OPS if op.name == name)
        row = dve_ops_mod._CUSTOM_DVE_ROW_BASE + len(dve_ops_mod.OPS)
        dve_ops_mod._SUB_OPCODE_FOR_NAME[name] = row
        shas = {}
        for ver in ("v3", "v4"):
            uops = lower(spec, ver=ver)
            shas[ver] = DveOpSpec(name=name, opcode=row, uops=uops,
                                  rd1_en=False).sha(ver)
        op = DveOp(name, spec, subdim=False, uops_sha=shas)
        dve_ops_mod.OPS.append(op)
        dve_ops_mod.CUSTOM_DVE_SPECS[name] = spec
        return op

    y1 = Src0 + One
    for _ in range(7):
        y1 = y1 * y1

    def _ref1(in0, in1, c0, c1, c2):
        y = 1.0 + in0
        for _ in range(7):
            y = y * y
        return y

    y2 = Src0
    for _ in range(6):
        y2 = y2 * y2

    def _ref2(in0, in1, c0, c1, c2):
        y = in0
        for _ in range(6):
            y = y * y
        return y

    _EXP_OPS = (_make("EXP128_ANT", y1, _ref1), _make("POW64_ANT", y2, _ref2))
    return _EXP_OPS

# stream_shuffle operates within each 32-partition quadrant (same mask for
# all quadrants). AV lhsT is interleaved [v(16)|ones(16)|v(16)|ones(16)] per
# head, so every quadrant is [out rows 0:16 | den rows 16:32]; the mask pulls
# each quadrant's 1/den rows onto its out rows (and keeps them at 16:32, so
# junk rows become den/den ~ 1).
_RC_SHUF = [16 + i for i in range(16)] + [16 + i for i in range(16)]


def build_kernel_body(tc, y_ap, x_ap, woutT_ap, bout_ap):
    """Emit the per-core attention program into TileContext tc.

    DRAM tensors:
      x_ap:     [256, 1792] bf16   (fused [w_qkv^T | x] per channel chunk:
                                    cols 0:768 = w_qkv^T with the q-part
                                    pre-scaled by SCALE/EXPN, cols 768:1792
                                    = one batch element channels x tokens,
                                    host-cast bf16)
      woutT_ap: [512, 256]  bf16   (w_out^T padded: 64-row blocks per head,
                                    interleaved [w(16)|0(16)|w(16)|0(16)])
      bout_ap:  [256, 1]    fp32
      y_ap:     [256, 1024] fp32 out
    """
    from contextlib import ExitStack
    from concourse import mybir

    nc = tc.nc
    f32 = mybir.dt.float32
    bf16 = mybir.dt.bfloat16

    with ExitStack() as ctx:
        singles = ctx.enter_context(tc.tile_pool(name="singles", bufs=1))
        evac = ctx.enter_context(tc.tile_pool(name="evac", bufs=2))
        exp_pool = ctx.enter_context(tc.tile_pool(name="exp", bufs=16))
        rc_pool = ctx.enter_context(tc.tile_pool(name="rc", bufs=3))
        sim_psum = ctx.enter_context(tc.tile_pool(name="simp", bufs=2, space="PSUM"))
        acc_psum = ctx.enter_context(tc.tile_pool(name="accp", bufs=2, space="PSUM"))

        # ---- phase 0: loads + constant prep ----
        # x (host-cast bf16) and wqkvT ride in ONE fused DMA per 128-channel
        # chunk: [128, 1024 x | 768 wq] -- halves the number of critical-path
        # DMA issues at kernel start.
        # the two chunks are issued from different HWDGE engines (SP and
        # ACT) so their transfers run on separate queues in parallel.
        xw = []
        for c in range(2):
            txw = singles.tile([128, N + 768], bf16, tag=f"xw_{c}")
            eng = nc.sync if c == 0 else nc.scalar
            eng.dma_start(out=txw, in_=x_ap[c * 128:(c + 1) * 128, :])
            xw.append(txw)

        def XB(c, lo, hi):      # x slice, channels c*128.., tokens lo:hi
            return xw[c][:, 768 + lo:768 + hi]

        def WQ(c, lo, hi):      # wqkvT slice, o-channels lo:hi
            return xw[c][:, lo:hi]

        wo = []
        for t in range(4):
            tw = singles.tile([128, 256], bf16, tag=f"wo_{t}")
            nc.sync.dma_start(out=tw, in_=woutT_ap[t * 128:(t + 1) * 128, :])
            wo.append(tw)
        bias = []
        for oc in range(2):
            tb = singles.tile([128, 1], f32, tag=f"bias_{oc}")
            nc.sync.dma_start(out=tb, in_=bout_ap[oc * 128:(oc + 1) * 128, :])
            bias.append(tb)

        # warmup during the ~7us input-DMA wait: a burst of matmuls on
        # memset data starts the PE p-state clock ramp, and a dummy exp
        # pulls the ~1.5-2.7us ACT spline-table load off the critical path.
        warm = singles.tile([128, 512], bf16, tag="warm")
        nc.gpsimd.memset(warm, 0.0)
        wps = acc_psum.tile([128, 512], f32, tag="acc", name="warmps")
        for i in range(6):
            nc.tensor.matmul(wps, warm[:, 0:128], warm,
                             start=(i == 0), stop=(i == 5))
        wexp = singles.tile([128, 8], bf16, tag="warmexp")
        nc.scalar.activation(out=wexp, in_=warm[:, 0:8],
                             func=mybir.ActivationFunctionType.Exp,
                             scale=EXPN)

        # vpack tiles: per jc, [128, 8*64] bf16. memset 1.0; v cols written by
        # strided copies from the vt GEMM. Column block for head h:
        # [v dims 0:16 | ones x16 | v dims 16:32 | ones x16].
        vpack = []
        for jc in range(8):
            tv = singles.tile([128, 512], bf16, tag=f"vpack_{jc}")
            nc.gpsimd.memset(tv, 1.0)
            vpack.append(tv)

        # out_all: final-GEMM rhs, 4 pair tiles x [128, 1024] bf16.
        # pair p = heads (2p, 2p+1): head A rows 0:64, head B rows 64:128,
        # each 64-block interleaved [out(16)|junk(16)|out(16)|junk(16)];
        # junk rows ~1.0 (den * 1/den) are killed by woutT's zero rows.
        out_all = []
        for t in range(4):
            ta = singles.tile([128, N], bf16, tag=f"out_all_{t}")
            out_all.append(ta)

        # ---- qkv GEMM helper: one o-chunk (q: oc 0,1; k: oc 2,3) ----
        qk = [None] * 4

        def emit_qkv_half(oc, nh):
            if qk[oc] is None:
                qk[oc] = singles.tile([128, N], bf16, tag=f"qk_{oc}",
                                      name=f"qk_{oc}")
            dst = qk[oc]
            ps = acc_psum.tile([128, 512], f32, tag="acc",
                               name=f"qkvps_{oc}_{nh}")
            for kc in range(2):
                nc.tensor.matmul(
                    ps,
                    WQ(kc, oc * 128, (oc + 1) * 128),
                    XB(kc, nh * 512, (nh + 1) * 512),
                    start=(kc == 0),
                    stop=(kc == 1),
                )
            nc.vector.tensor_copy(out=dst[:, nh * 512:(nh + 1) * 512], in_=ps)

        def emit_qkv(oc):
            for nh in range(2):
                emit_qkv_half(oc, nh)

        # ---- vT GEMM + packing: vt[jc] = x[:, jc]^T @ w_v^T ----
        def emit_vt(jc):
            ps = acc_psum.tile([128, 256], f32, tag="acc")
            for kc in range(2):
                nc.tensor.matmul(
                    ps,
                    XB(kc, jc * 128, (jc + 1) * 128),
                    WQ(kc, 512, 768),
                    start=(kc == 0),
                    stop=(kc == 1),
                )
            pv = ps[:, :].rearrange("p (h c) -> p h c", c=32)
            vv = vpack[jc][:, :].rearrange("p (h c) -> p h c", c=64)
            nc.vector.tensor_copy(out=vv[:, :, 0:16], in_=pv[:, :, 0:16])
            nc.vector.tensor_copy(out=vv[:, :, 32:48], in_=pv[:, :, 16:32])

        # quad-0 nh0 halves ONLY before the first sims: group (0,0)'s jc 0-3
        # need just k cols 0:512 and q cols 0:512, so sim (and with it the
        # ACT exp stream) starts after 4 matmuls + 2 casts instead of 8+4.
        # The nh1 halves follow inside group 0 (at jc=1, well before their
        # first consumer at jc=4); vt and the quad-1 weights at jc=3. All of
        # these acc-pool psum allocations must precede group 0's first AV
        # allocation (at jc=3) -- an AV accumulator holds its ring slot
        # until normalize, so later requests would deadlock the ring.
        emit_qkv_half(2, 0)   # k heads 0-3, tokens 0:512
        emit_qkv_half(0, 0)   # q heads 0-3, tokens 0:512

        exp_op1, exp_op2 = _register_exp_ops()

        # projection helper: one (oc, nh) unit = 4 accumulating matmuls +
        # bias + store. nh=0 runs early (after group 2); nh=1 in the tail.
        def emit_proj(oc, nh, ps=None):
            if ps is None:
                ps = acc_psum.tile([128, 512], f32, tag="acc")
            for t in range(4):
                nc.tensor.matmul(
                    ps,
                    wo[t][:, oc * 128:(oc + 1) * 128],
                    out_all[t][:, nh * 512:(nh + 1) * 512],
                    start=(t == 0),
                    stop=(t == 3),
                )
            ys = evac.tile([128, 512], bf16, tag="y")
            nc.vector.tensor_scalar_add(out=ys, in0=ps, scalar1=bias[oc])
            nc.sync.dma_start(
                out=y_ap[oc * 128:(oc + 1) * 128, nh * 512:(nh + 1) * 512],
                in_=ys,
            )

        # ---- main loop: sim^T -> exp -> AV(+den) -> normalize ----
        # group g = (Q, ih); 32 units of [128,512] per group; exp tiles hold
        # 3 units (final tile of a group holds 2). Groups are SOFTWARE
        # PIPELINED: group g's AV tail (jc 5-7) and normalize are emitted
        # inside group g+1's jc loop, so PE never drains at group ends while
        # ACT finishes the last exp tiles.
        GROUPS = ((0, 0), (1, 0), (0, 1), (1, 1))
        g_exp = [dict() for _ in GROUPS]    # unit u -> (exp_tile, slot)
        g_acc = [dict() for _ in GROUPS]    # pq -> acc psum tile

        def emit_av(gi, jc):
            Q, ih = GROUPS[gi]
            for pq in range(2):
                if jc == 0:
                    g_acc[gi][pq] = acc_psum.tile(
                        [128, 512], f32, tag="acc", name=f"av_{gi}_{pq}")
                at = g_acc[gi][pq]
                for ab in range(2):
                    hq = 2 * pq + ab
                    hg = 4 * Q + hq
                    et, s = g_exp[gi][jc * 4 + hq]
                    nc.tensor.matmul(
                        at[64 * ab:64 * ab + 64, :],
                        vpack[jc][:, 64 * hg:64 * hg + 64],
                        et[:, s * 512:(s + 1) * 512],
                        start=(jc == 0),
                        stop=(jc == 7),
                        tile_position=(0, 64 * ab),
                        skip_group_check=True,
                    )

        def emit_norm(gi):
            Q, ih = GROUPS[gi]
            for pq in range(2):
                pair = 2 * Q + pq
                at = g_acc[gi][pq]
                rc = rc_pool.tile([128, 512], f32, tag="rc",
                                  name=f"rc_{pair}_{ih}")
                nc.vector.reciprocal_approx_fast(out=rc, in_=at[:, :])
                rca = rc_pool.tile([128, 512], f32, tag="rca",
                                   name=f"rca_{pair}_{ih}")
                nc.vector.stream_shuffle(out=rca, in_=rc, mask=_RC_SHUF)
                nc.vector.tensor_mul(
                    out=out_all[pair][:, ih * 512:(ih + 1) * 512],
                    in0=at[:, :], in1=rca)

        for gi, (Q, ih) in enumerate(GROUPS):
            cur = {"psum": None, "exp": None, "units": 0, "start_u": 0}

            def flush():
                if cur["psum"] is None:
                    return
                w = cur["units"] * 512
                # two mid-group tiles go to the DVE via the 2-pass
                # (1+s)^8192 chain; the rest to ACT (exact exp, recovering
                # x via the free affine scale). Group 0's tile grid is
                # shifted by its 4 single-unit warmup flushes.
                if cur["start_u"] in ((10, 22) if gi == 0 else (9, 21)):
                    scr = rc_pool.tile([128, 1536], f32, tag="escr",
                                       name=f"escr_{gi}_{cur['start_u']}")
                    nc.vector._custom_dve(
                        exp_op1,
                        out=scr[:, 0:w],
                        in0=cur["psum"][:, 0:w],
                    )
                    nc.vector._custom_dve(
                        exp_op2,
                        out=cur["exp"][:, 0:w],
                        in0=scr[:, 0:w],
                    )
                else:
                    nc.scalar.activation(
                        out=cur["exp"][:, 0:w],
                        in_=cur["psum"][:, 0:w],
                        func=mybir.ActivationFunctionType.Exp,
                        scale=EXPN,
                    )
                cur["psum"] = None
                cur["exp"] = None
                cur["units"] = 0

            for jc in range(8):
                if gi == 0 and jc == 1:
                    emit_qkv_half(2, 1)   # k heads 0-3, tokens 512:1024
                    emit_qkv_half(0, 1)   # q heads 0-3, tokens 512:1024
                if gi == 0 and jc == 3:
                    # PE is 2 tiles ahead of ACT here -- spend the slack
                    # on vt + quad-1 qkv weights (see ring note above).
                    for vjc in range(8):
                        emit_vt(vjc)
                    emit_qkv(3)   # k heads 4-7
                    emit_qkv(1)   # q heads 4-7
                for hq in range(4):
                    u = jc * 4 + hq
                    if cur["psum"] is None:
                        cur["psum"] = sim_psum.tile(
                            [128, 1536], f32, tag="sim",
                            name=f"sim_{gi}_{u}")
                        cur["exp"] = exp_pool.tile(
                            [128, 1536], bf16, tag="exp",
                            name=f"exp_{gi}_{u}")
                        cur["start_u"] = u
                    s = cur["units"]
                    nc.tensor.matmul(
                        cur["psum"][:, s * 512:(s + 1) * 512],
                        qk[2 + Q][32 * hq:32 * (hq + 1),
                                  jc * 128:(jc + 1) * 128],
                        qk[Q][32 * hq:32 * (hq + 1),
                              ih * 512:(ih + 1) * 512],
                        start=True,
                        stop=True,
                        tile_position=(32 * hq, 0),
                    )
                    g_exp[gi][u] = (cur["exp"], s)
                    cur["units"] += 1
                    # group 0's first jc flushes per unit so ACT spins up
                    # ~3us earlier; afterwards tiles hold 3 units.
                    if cur["units"] == 3 or u == 31 or (gi == 0 and jc == 0):
                        flush()
                # previous group's AV tail + normalize, pipelined into
                # this group's first jc slots
                if gi > 0 and jc <= 2:
                    emit_av(gi - 1, jc + 5)
                    if gi == 3 and jc <= 1:
                        # nh=0 projection: its out_all inputs (groups 0-1)
                        # finished long ago. One unit per jc slot (one
                        # sim-ring steal at a time), emitted BEFORE the
                        # norm chain so its bias-add isn't queued behind
                        # it on DVE; stores overlap the rest of group 3.
                        pp = sim_psum.tile([128, 1536], f32, tag="sim",
                                           name=f"projp_{jc}")
                        emit_proj(jc, 0, pp[:, 0:512])
                    if jc == 2:
                        emit_norm(gi - 1)
                if jc >= 3:
                    emit_av(gi, jc - 3)

        # ---- drain the pipeline: last group's AV tail + normalize ----
        for jc in range(5, 8):
            emit_av(3, jc)
        emit_norm(3)

        # ---- tail projection (nh=1; nh=0 ran early during group 3) ----
        # psum borrows idle sim-ring slots: the acc ring is still held by
        # group 3's AV accumulators until the normalize muls run, which
        # would serialize proj behind the whole DVE chain. On the sim ring
        # the t=0,1 accumulation steps (inputs ready since group 3's jc2)
        # overlap the normalize instead.
        for oc in range(2):
            pp = sim_psum.tile([128, 1536], f32, tag="sim",
                               name=f"projt_{oc}")
            emit_proj(oc, 1, pp[:, 0:512])


def _prep_weights(w_qkv, w_out, b_out):
    """Host-side weight preparation (numpy)."""
    wq = w_qkv.astype(np.float32).copy()
    # fold softmax scale AND the 1/EXPN logit pre-scale into w_q
    wq[0:DIM_HEAD] *= SCALE / EXPN
    wqkvT = np.ascontiguousarray(wq.T).astype(_BF16)          # [256, 768]

    w_outT = np.ascontiguousarray(w_out.astype(np.float32).T)  # [hd, o]
    # interleaved pad matching the AV lhsT layout: per head-block of 64 rows,
    # [w dims 0:16 | zeros x16 | w dims 16:32 | zeros x16]
    pad = np.zeros((8, 4, 16, DIM), dtype=np.float32)
    for h in range(NUM_HEADS):
        pad[h, 0, :, :] = w_outT[h * D:h * D + 16, :]
        pad[h, 2, :, :] = w_outT[h * D + 16:h * D + 32, :]
    woutT = pad.reshape(512, DIM).astype(_BF16)               # [512, 256]

    bout = b_out.astype(np.float32).reshape(DIM, 1)           # [256, 1]
    return wqkvT, woutT, bout


def _build_program():
    global _PROGRAM
    if _PROGRAM is not None:
        return _PROGRAM
    import concourse.tile as tile
    from concourse import bacc, mybir

    nc = bacc.Bacc("TRN2", target_bir_lowering=False, debug=False,
                   num_devices=NCORES)
    xw_ap = nc.dram_tensor("xw", [DIM, N + 3 * DIM_HEAD], mybir.dt.bfloat16,
                           kind="ExternalInput").ap()
    woutT_ap = nc.dram_tensor("woutT", [512, DIM], mybir.dt.bfloat16,
                              kind="ExternalInput").ap()
    bout_ap = nc.dram_tensor("bout", [DIM, 1], mybir.dt.float32,
                             kind="ExternalInput").ap()
    y_ap = nc.dram_tensor("y", [DIM, N], mybir.dt.bfloat16,
                          kind="ExternalOutput").ap()
    with tile.TileContext(nc) as tc:
        build_kernel_body(tc, y_ap, xw_ap, woutT_ap, bout_ap)
    nc.compile()
    _PROGRAM = nc
    return nc


def kernel(x, w_qkv, w_out, b_out, trace=False):
    """Full-input entry point: shard over batch, run on 8 cores, gather."""
    from concourse import bass_utils

    nc = _build_program()
    wqkvT, woutT, bout = _prep_weights(w_qkv, w_out, b_out)
    in_maps = []
    for b in range(B):
        xb16 = np.asarray(x[b], dtype=np.float32).reshape(DIM, N).astype(_BF16)
        in_maps.append({
            "xw": np.ascontiguousarray(np.concatenate([wqkvT, xb16], axis=1)),
            "woutT": woutT,
            "bout": bout,
        })
    res = bass_utils.run_bass_kernel_spmd(
        nc, in_maps, core_ids=list(range(NCORES)), trace=trace)
    y = np.stack([np.asarray(res.results[b]["y"], dtype=np.float32)
                  .reshape(DIM, H, W) for b in range(B)])
    kernel.last_results = res
    return y


# revision 65
# speedup vs baseline: 1.1999x; 1.0463x over previous
"""Trainium2 Bass kernel for nn_Attention2D (B=8, C=256, H=W=32, 8 heads, d=32).

Strategy: data-parallel over batch, one batch element per NeuronCore (8 cores).

Per-core pipeline (n = H*W = 1024 tokens, head dim d = 32):
  phase 0: load x [256,1024] fp32 -> bf16 (one cast on DVE, one on ACT);
           load host-prepped weights.
  qkv:     k = w_k @ x, q = (scale*w_q) @ x  ([256,1024] head-major, bf16,
           quad-0 chunks first so sim can start early)
  vpack:   vt[jc] = x[:, jc]^T @ w_v^T packed per head as
           [v(16)|ones(16)|v(16)|ones(16)] -> [128, 8*64] bf16. The ones
           columns make the AV matmul emit the softmax denominator for free.
  sim^T:   per (head, j-chunk): matmul(lhsT=k slice [32,128], rhs=q slice
           [32,512]) -> PSUM ring tiles [128,1536] (3 units); 4 heads
           coreside via row quadrants (K=32 packing).
  exp:     ACT Exp over the 3-unit PSUM tiles -> bf16 SBUF (max-subtraction
           skipped: logits ~N(0,0.8), measured max |sim| < 5; exp safe).
  AV+den:  per (pair, ih, jc): 2 matmuls (head A rows 0:64 at tile_position
           (0,0), head B rows 64:128 at (0,64)); lhsT [128,64] =
           [v(16)|ones(16)|v(16)|ones(16)] so every 32-row quadrant holds
           16 out rows + 16 denominator rows. AV is interleaved into the
           sim loop (lag 3 j-chunks) to fill PE gaps while ACT paces the
           sim ring.
  norm:    rc = reciprocal_approx_fast(acc) (full tile); stream_shuffle
           aligns 1/den onto the out rows; one full-tile multiply writes
           out_all. Junk rows become den/den ~ 1.0 and are killed by the
           zero rows of the padded projection weights.
  proj:    final = w_outT_padded^T @ out_all + b_out -> y [256,1024] fp32.
"""

import numpy as np
import ml_dtypes

B, DIM, H, W = 8, 256, 32, 32
NUM_HEADS = 8
DIM_HEAD = 256
D = DIM_HEAD // NUM_HEADS          # 32 per-head dim
N = H * W                          # 1024 tokens
SCALE = (DIM_HEAD / NUM_HEADS) ** (-0.5)
NCORES = 8

_BF16 = ml_dtypes.bfloat16

_PROGRAM = None  # compiled Bass program cache (one per process)

# DVE exp offload: logits are pre-scaled by 1/EXPN (folded into w_q on the
# host); the ACT engine recovers exp(x) via its free affine (scale=EXPN) and
# the DVE computes exp(x) ~ (1 + x/8192)^8192 in TWO chained custom ops:
# (1+s)^128 (1 add + 7 squarings, exactly the 8-stage v3 pipeline) followed
# by y^64 (6 squarings). Max rel err 0.3% over |x|<=6 -- bf16-noise level.
EXPN = 8192.0
_EXP_OPS = None


def _register_exp_ops():
    """Register custom-DVE ops EXP128_ANT ((1+x)^128) and POW64_ANT (x^64)."""
    global _EXP_OPS
    if _EXP_OPS is not None:
        return _EXP_OPS
    import concourse.dve_ops as dve_ops_mod
    from concourse.dve_spec import Spec, Src0, One, lower
    from concourse.dve_uop import DveOpSpec
    from concourse.dve_ops import DveOp

    def _make(name, body, ref):
        spec = Spec(body=body, reference=ref)
        if name in dve_ops_mod._SUB_OPCODE_FOR_NAME:
            return next(op for op in dve_ops_mod.OPS if op.name == name)
        row = dve_ops_mod._CUSTOM_DVE_ROW_BASE + len(dve_ops_mod.OPS)
        dve_ops_mod._SUB_OPCODE_FOR_NAME[name] = row
        shas = {}
        for ver in ("v3", "v4"):
            uops = lower(spec, ver=ver)
            shas[ver] = DveOpSpec(name=name, opcode=row, uops=uops,
                                  rd1_en=False).sha(ver)
        op = DveOp(name, spec, subdim=False, uops_sha=shas)
        dve_ops_mod.OPS.append(op)
        dve_ops_mod.CUSTOM_DVE_SPECS[name] = spec
        return op

    y1 = Src0 + One
    for _ in range(7):
        y1 = y1 * y1

    def _ref1(in0, in1, c0, c1, c2):
        y = 1.0 + in0
        for _ in range(7):
            y = y * y
        return y

    y2 = Src0
    for _ in range(6):
        y2 = y2 * y2

    def _ref2(in0, in1, c0, c1, c2):
        y = in0
        for _ in range(6):
            y = y * y
        return y

    _EXP_OPS = (_make("EXP128_ANT", y1, _ref1), _make("POW64_ANT", y2, _ref2))
    return _EXP_OPS

# stream_shuffle operates within each 32-partition quadrant (same mask for
# all quadrants). AV lhsT is interleaved [v(16)|ones(16)|v(16)|ones(16)] per
# head, so every quadrant is [out rows 0:16 | den rows 16:32]; the mask pulls
# each quadrant's 1/den rows onto its out rows (and keeps them at 16:32, so
# junk rows become den/den ~ 1).
_RC_SHUF = [16 + i for i in range(16)] + [16 + i for i in range(16)]


def build_kernel_body(tc, y_ap, x_ap, woutT_ap, bout_ap):
    """Emit the per-core attention program into TileContext tc.

    DRAM tensors:
      x_ap:     [256, 1792] bf16   (fused [w_qkv^T | x] per channel chunk:
                                    cols 0:768 = w_qkv^T with the q-part
                                    pre-scaled by SCALE/EXPN, cols 768:1792
                                    = one batch element channels x tokens,
                                    host-cast bf16)
      woutT_ap: [512, 256]  bf16   (w_out^T padded: 64-row blocks per head,
                                    interleaved [w(16)|0(16)|w(16)|0(16)])
      bout_ap:  [256, 1]    fp32
      y_ap:     [256, 1024] fp32 out
    """
    from contextlib import ExitStack
    from concourse import mybir

    nc = tc.nc
    f32 = mybir.dt.float32
    bf16 = mybir.dt.bfloat16

    with ExitStack() as ctx:
        singles = ctx.enter_context(tc.tile_pool(name="singles", bufs=1))
        evac = ctx.enter_context(tc.tile_pool(name="evac", bufs=2))
        exp_pool = ctx.enter_context(tc.tile_pool(name="exp", bufs=16))
        rc_pool = ctx.enter_context(tc.tile_pool(name="rc", bufs=3))
        sim_psum = ctx.enter_context(tc.tile_pool(name="simp", bufs=2, space="PSUM"))
        acc_psum = ctx.enter_context(tc.tile_pool(name="accp", bufs=2, space="PSUM"))

        # ---- phase 0: loads + constant prep ----
        # x (host-cast bf16) and wqkvT ride in ONE fused DMA per 128-channel
        # chunk: [128, 1024 x | 768 wq] -- halves the number of critical-path
        # DMA issues at kernel start.
        # the two chunks are issued from different HWDGE engines (SP and
        # ACT) so their transfers run on separate queues in parallel.
        xw = []
        for c in range(2):
            txw = singles.tile([128, N + 768], bf16, tag=f"xw_{c}")
            eng = nc.sync if c == 0 else nc.scalar
            eng.dma_start(out=txw, in_=x_ap[c * 128:(c + 1) * 128, :])
            xw.append(txw)

        def XB(c, lo, hi):      # x slice, channels c*128.., tokens lo:hi
            return xw[c][:, 768 + lo:768 + hi]

        def WQ(c, lo, hi):      # wqkvT slice, o-channels lo:hi
            return xw[c][:, lo:hi]

        wo = []
        for t in range(4):
            tw = singles.tile([128, 256], bf16, tag=f"wo_{t}")
            nc.sync.dma_start(out=tw, in_=woutT_ap[t * 128:(t + 1) * 128, :])
            wo.append(tw)
        bias = []
        for oc in range(2):
            tb = singles.tile([128, 1], f32, tag=f"bias_{oc}")
            nc.sync.dma_start(out=tb, in_=bout_ap[oc * 128:(oc + 1) * 128, :])
            bias.append(tb)

        # warmup during the ~7us input-DMA wait: a burst of matmuls on
        # memset data starts the PE p-state clock ramp, and a dummy exp
        # pulls the ~1.5-2.7us ACT spline-table load off the critical path.
        warm = singles.tile([128, 512], bf16, tag="warm")
        nc.gpsimd.memset(warm, 0.0)
        wps = acc_psum.tile([128, 512], f32, tag="acc", name="warmps")
        for i in range(6):
            nc.tensor.matmul(wps, warm[:, 0:128], warm,
                             start=(i == 0), stop=(i == 5))
        wexp = singles.tile([128, 8], bf16, tag="warmexp")
        nc.scalar.activation(out=wexp, in_=warm[:, 0:8],
                             func=mybir.ActivationFunctionType.Exp,
                             scale=EXPN)

        # vpack tiles: per jc, [128, 8*64] bf16. memset 1.0; v cols written by
        # strided copies from the vt GEMM. Column block for head h:
        # [v dims 0:16 | ones x16 | v dims 16:32 | ones x16].
        vpack = []
        for jc in range(8):
            tv = singles.tile([128, 512], bf16, tag=f"vpack_{jc}")
            nc.gpsimd.memset(tv, 1.0)
            vpack.append(tv)

        # out_all: final-GEMM rhs, 4 pair tiles x [128, 1024] bf16.
        # pair p = heads (2p, 2p+1): head A rows 0:64, head B rows 64:128,
        # each 64-block interleaved [out(16)|junk(16)|out(16)|junk(16)];
        # junk rows ~1.0 (den * 1/den) are killed by woutT's zero rows.
        out_all = []
        for t in range(4):
            ta = singles.tile([128, N], bf16, tag=f"out_all_{t}")
            out_all.append(ta)

        # ---- qkv GEMM helper: one o-chunk (q: oc 0,1; k: oc 2,3) ----
        qk = [None] * 4

        def emit_qkv_half(oc, nh):
            if qk[oc] is None:
                qk[oc] = singles.tile([128, N], bf16, tag=f"qk_{oc}",
                                      name=f"qk_{oc}")
            dst = qk[oc]
            ps = acc_psum.tile([128, 512], f32, tag="acc",
                               name=f"qkvps_{oc}_{nh}")
            for kc in range(2):
                nc.tensor.matmul(
                    ps,
                    WQ(kc, oc * 128, (oc + 1) * 128),
                    XB(kc, nh * 512, (nh + 1) * 512),
                    start=(kc == 0),
                    stop=(kc == 1),
                )
            nc.vector.tensor_copy(out=dst[:, nh * 512:(nh + 1) * 512], in_=ps)

        def emit_qkv(oc):
            for nh in range(2):
                emit_qkv_half(oc, nh)

        # ---- vT GEMM + packing: vt[jc] = x[:, jc]^T @ w_v^T ----
        def emit_vt(jc):
            ps = acc_psum.tile([128, 256], f32, tag="acc")
            for kc in range(2):
                nc.tensor.matmul(
                    ps,
                    XB(kc, jc * 128, (jc + 1) * 128),
                    WQ(kc, 512, 768),
                    start=(kc == 0),
                    stop=(kc == 1),
                )
            pv = ps[:, :].rearrange("p (h c) -> p h c", c=32)
            vv = vpack[jc][:, :].rearrange("p (h c) -> p h c", c=64)
            nc.vector.tensor_copy(out=vv[:, :, 0:16], in_=pv[:, :, 0:16])
            nc.vector.tensor_copy(out=vv[:, :, 32:48], in_=pv[:, :, 16:32])

        # quad-0 nh0 halves ONLY before the first sims: group (0,0)'s jc 0-3
        # need just k cols 0:512 and q cols 0:512, so sim (and with it the
        # ACT exp stream) starts after 4 matmuls + 2 casts instead of 8+4.
        # The nh1 halves follow inside group 0 (at jc=1, well before their
        # first consumer at jc=4); vt and the quad-1 weights at jc=3. All of
        # these acc-pool psum allocations must precede group 0's first AV
        # allocation (at jc=3) -- an AV accumulator holds its ring slot
        # until normalize, so later requests would deadlock the ring.
        emit_qkv_half(2, 0)   # k heads 0-3, tokens 0:512
        emit_qkv_half(0, 0)   # q heads 0-3, tokens 0:512

        exp_op1, exp_op2 = _register_exp_ops()

        # projection helper: one (oc, nh) unit = 4 accumulating matmuls +
        # bias + store. nh=0 runs early (after group 2); nh=1 in the tail.
        def emit_proj(oc, nh, ps=None):
            if ps is None:
                ps = acc_psum.tile([128, 512], f32, tag="acc")
            for t in range(4):
                nc.tensor.matmul(
                    ps,
                    wo[t][:, oc * 128:(oc + 1) * 128],
                    out_all[t][:, nh * 512:(nh + 1) * 512],
                    start=(t == 0),
                    stop=(t == 3),
                )
            ys = evac.tile([128, 512], bf16, tag="y")
            nc.vector.tensor_scalar_add(out=ys, in0=ps, scalar1=bias[oc])
            # tail (nh=1) stores issue from the idle ACT queue so the two
            # final DMAs don't serialize their issue on sync; nh=0 stores
            # stay on sync (ACT is mid-exp-stream when they fire).
            eng = nc.scalar if nh == 1 else nc.sync
            eng.dma_start(
                out=y_ap[oc * 128:(oc + 1) * 128, nh * 512:(nh + 1) * 512],
                in_=ys,
            )

        # ---- main loop: sim^T -> exp -> AV(+den) -> normalize ----
        # group g = (Q, ih); 32 units of [128,512] per group; exp tiles hold
        # 3 units (final tile of a group holds 2). Groups are SOFTWARE
        # PIPELINED: group g's AV tail (jc 5-7) and normalize are emitted
        # inside group g+1's jc loop, so PE never drains at group ends while
        # ACT finishes the last exp tiles.
        GROUPS = ((0, 0), (1, 0), (0, 1), (1, 1))
        g_exp = [dict() for _ in GROUPS]    # unit u -> (exp_tile, slot)
        g_acc = [dict() for _ in GROUPS]    # pq -> acc psum tile

        def emit_av(gi, jc):
            Q, ih = GROUPS[gi]
            for pq in range(2):
                if jc == 0:
                    g_acc[gi][pq] = acc_psum.tile(
                        [128, 512], f32, tag="acc", name=f"av_{gi}_{pq}")
                at = g_acc[gi][pq]
                for ab in range(2):
                    hq = 2 * pq + ab
                    hg = 4 * Q + hq
                    et, s = g_exp[gi][jc * 4 + hq]
                    nc.tensor.matmul(
                        at[64 * ab:64 * ab + 64, :],
                        vpack[jc][:, 64 * hg:64 * hg + 64],
                        et[:, s * 512:(s + 1) * 512],
                        start=(jc == 0),
                        stop=(jc == 7),
                        tile_position=(0, 64 * ab),
                        skip_group_check=True,
                    )

        def emit_norm(gi):
            Q, ih = GROUPS[gi]
            for pq in range(2):
                pair = 2 * Q + pq
                at = g_acc[gi][pq]
                rc = rc_pool.tile([128, 512], f32, tag="rc",
                                  name=f"rc_{pair}_{ih}")
                nc.vector.reciprocal_approx_fast(out=rc, in_=at[:, :])
                rca = rc_pool.tile([128, 512], f32, tag="rca",
                                   name=f"rca_{pair}_{ih}")
                nc.vector.stream_shuffle(out=rca, in_=rc, mask=_RC_SHUF)
                nc.vector.tensor_mul(
                    out=out_all[pair][:, ih * 512:(ih + 1) * 512],
                    in0=at[:, :], in1=rca)

        for gi, (Q, ih) in enumerate(GROUPS):
            cur = {"psum": None, "exp": None, "units": 0, "start_u": 0}

            def flush():
                if cur["psum"] is None:
                    return
                w = cur["units"] * 512
                # two mid-group tiles go to the DVE via the 2-pass
                # (1+s)^8192 chain; the rest to ACT (exact exp, recovering
                # x via the free affine scale). Group 0's tile grid is
                # shifted by its 4 single-unit warmup flushes.
                if cur["start_u"] in ((10, 22) if gi == 0 else (9, 21)):
                    scr = rc_pool.tile([128, 1536], f32, tag="escr",
                                       name=f"escr_{gi}_{cur['start_u']}")
                    nc.vector._custom_dve(
                        exp_op1,
                        out=scr[:, 0:w],
                        in0=cur["psum"][:, 0:w],
                    )
                    nc.vector._custom_dve(
                        exp_op2,
                        out=cur["exp"][:, 0:w],
                        in0=scr[:, 0:w],
                    )
                else:
                    nc.scalar.activation(
                        out=cur["exp"][:, 0:w],
                        in_=cur["psum"][:, 0:w],
                        func=mybir.ActivationFunctionType.Exp,
                        scale=EXPN,
                    )
                cur["psum"] = None
                cur["exp"] = None
                cur["units"] = 0

            for jc in range(8):
                if gi == 0 and jc == 1:
                    emit_qkv_half(2, 1)   # k heads 0-3, tokens 512:1024
                    emit_qkv_half(0, 1)   # q heads 0-3, tokens 512:1024
                if gi == 0 and jc == 3:
                    # PE is 2 tiles ahead of ACT here -- spend the slack
                    # on vt + quad-1 qkv weights (see ring note above).
                    for vjc in range(8):
                        emit_vt(vjc)
                    emit_qkv(3)   # k heads 4-7
                    emit_qkv(1)   # q heads 4-7
                for hq in range(4):
                    u = jc * 4 + hq
                    if cur["psum"] is None:
                        cur["psum"] = sim_psum.tile(
                            [128, 1536], f32, tag="sim",
                            name=f"sim_{gi}_{u}")
                        cur["exp"] = exp_pool.tile(
                            [128, 1536], bf16, tag="exp",
                            name=f"exp_{gi}_{u}")
                        cur["start_u"] = u
                    s = cur["units"]
                    nc.tensor.matmul(
                        cur["psum"][:, s * 512:(s + 1) * 512],
                        qk[2 + Q][32 * hq:32 * (hq + 1),
                                  jc * 128:(jc + 1) * 128],
                        qk[Q][32 * hq:32 * (hq + 1),
                              ih * 512:(ih + 1) * 512],
                        start=True,
                        stop=True,
                        tile_position=(32 * hq, 0),
                    )
                    g_exp[gi][u] = (cur["exp"], s)
                    cur["units"] += 1
                    # group 0's first jc flushes per unit so ACT spins up
                    # ~3us earlier; afterwards tiles hold 3 units.
                    if cur["units"] == 3 or u == 31 or (gi == 0 and jc == 0):
                        flush()
                # previous group's AV tail + normalize, pipelined into
                # this group's first jc slots
                if gi > 0 and jc <= 2:
                    emit_av(gi - 1, jc + 5)
                    if gi == 3 and jc <= 1:
                        # nh=0 projection: its out_all inputs (groups 0-1)
                        # finished long ago. One unit per jc slot (one
                        # sim-ring steal at a time), emitted BEFORE the
                        # norm chain so its bias-add isn't queued behind
                        # it on DVE; stores overlap the rest of group 3.
                        pp = sim_psum.tile([128, 1536], f32, tag="sim",
                                           name=f"projp_{jc}")
                        emit_proj(jc, 0, pp[:, 0:512])
                    if jc == 2:
                        emit_norm(gi - 1)
                if jc >= 3:
                    emit_av(gi, jc - 3)

        # ---- drain the pipeline: last group's AV tail + normalize ----
        for jc in range(5, 8):
            emit_av(3, jc)
        emit_norm(3)

        # ---- tail projection (nh=1; nh=0 ran early during group 3) ----
        # psum borrows idle sim-ring slots: the acc ring is still held by
        # group 3's AV accumulators until the normalize muls run, which
        # would serialize proj behind the whole DVE chain. On the sim ring
        # the t=0,1 accumulation steps (inputs ready since group 3's jc2)
        # overlap the normalize instead.
        for oc in range(2):
            pp = sim_psum.tile([128, 1536], f32, tag="sim",
                               name=f"projt_{oc}")
            emit_proj(oc, 1, pp[:, 0:512])


def _prep_weights(w_qkv, w_out, b_out):
    """Host-side weight preparation (numpy)."""
    wq = w_qkv.astype(np.float32).copy()
    # fold softmax scale AND the 1/EXPN logit pre-scale into w_q
    wq[0:DIM_HEAD] *= SCALE / EXPN
    wqkvT = np.ascontiguousarray(wq.T).astype(_BF16)          # [256, 768]

    w_outT = np.ascontiguousarray(w_out.astype(np.float32).T)  # [hd, o]
    # interleaved pad matching the AV lhsT layout: per head-block of 64 rows,
    # [w dims 0:16 | zeros x16 | w dims 16:32 | zeros x16]
    pad = np.zeros((8, 4, 16, DIM), dtype=np.float32)
    for h in range(NUM_HEADS):
        pad[h, 0, :, :] = w_outT[h * D:h * D + 16, :]
        pad[h, 2, :, :] = w_outT[h * D + 16:h * D + 32, :]
    woutT = pad.reshape(512, DIM).astype(_BF16)               # [512, 256]

    bout = b_out.astype(np.float32).reshape(DIM, 1)           # [256, 1]
    return wqkvT, woutT, bout


def _build_program():
    global _PROGRAM
    if _PROGRAM is not None:
        return _PROGRAM
    import concourse.tile as tile
    from concourse import bacc, mybir

    nc = bacc.Bacc("TRN2", target_bir_lowering=False, debug=False,
                   num_devices=NCORES)
    xw_ap = nc.dram_tensor("xw", [DIM, N + 3 * DIM_HEAD], mybir.dt.bfloat16,
                           kind="ExternalInput").ap()
    woutT_ap = nc.dram_tensor("woutT", [512, DIM], mybir.dt.bfloat16,
                              kind="ExternalInput").ap()
    bout_ap = nc.dram_tensor("bout", [DIM, 1], mybir.dt.float32,
                             kind="ExternalInput").ap()
    y_ap = nc.dram_tensor("y", [DIM, N], mybir.dt.bfloat16,
                          kind="ExternalOutput").ap()
    with tile.TileContext(nc) as tc:
        build_kernel_body(tc, y_ap, xw_ap, woutT_ap, bout_ap)
    nc.compile()
    _PROGRAM = nc
    return nc


def kernel(x, w_qkv, w_out, b_out, trace=False):
    """Full-input entry point: shard over batch, run on 8 cores, gather."""
    from concourse import bass_utils

    nc = _build_program()
    wqkvT, woutT, bout = _prep_weights(w_qkv, w_out, b_out)
    in_maps = []
    for b in range(B):
        xb16 = np.asarray(x[b], dtype=np.float32).reshape(DIM, N).astype(_BF16)
        in_maps.append({
            "xw": np.ascontiguousarray(np.concatenate([wqkvT, xb16], axis=1)),
            "woutT": woutT,
            "bout": bout,
        })
    res = bass_utils.run_bass_kernel_spmd(
        nc, in_maps, core_ids=list(range(NCORES)), trace=trace)
    y = np.stack([np.asarray(res.results[b]["y"], dtype=np.float32)
                  .reshape(DIM, H, W) for b in range(B)])
    kernel.last_results = res
    return y
